# revision 45
# baseline (speedup 1.0000x reference)
"""Trainium2 Bass kernel for CustomPositionsPiecewiseConv2d.

Math: for knots positions=[-1,-.5,0,.5,1] and x in [0,1], only the last two
spline intervals are active.  With g2 = 2*min(v,0.5) and c4 = relu(2v-1)
(bf16 rounding absorbs the isclose(v,1) mask), the per-tap contribution is
    contrib = W3 + (1-g2)*(W2-W3) + c4*(W4-W3)
            = W2 + g2*(W3-W2) + c4*(W4-W3)
so  out = G2 (x) (W3-W2) + C4 (x) (W4-W3) + bias + sum_ck W2.

GEMM packing: contraction over (2 planes x 32 ch x 9 taps) = 576 lanes.
Per image one Y buffer [128, 66, 66] holds [g2; c4; g2 row+1; c4 row+1];
a K=128 matmul then contracts taps (0,kw) and (1,kw) simultaneously
(kw = read-time column offset), and the kh=2 row uses K=64 matmuls on the
lower half: 6 matmuls per output tile instead of 9 (or 18 split).  All
plane->Y copies are full-width row windows (contiguous DMA descriptors).

Output is written [O, IPC, H, W] (per-partition-contiguous 4KB segments)
and transposed on host.  Sharding: data-parallel, 2 images per core.
"""

import numpy as np

B, C, H, W = 16, 32, 64, 64
O, P, KH, KW = 128, 5, 3, 3
NCORES = 8
IPC = B // NCORES            # images per core
HP, WP = H + 2, W + 2        # padded plane (pad=1)
RT = 8                       # output rows per L-tile
NT = H // RT                 # L-tiles per image
K2 = KH * KW
NS = 6                       # stationary steps: 3 row-pair + 3 single (zero-padded K)
ATOL = 1e-5
RTOL = 1e-5

# x row chunks for the load->phi->gather->matmul pipeline
RC = [(0, 18), (18, 49), (49, 64)]
# gather chunks: disjoint ((lo0,lo1),(hi0,hi1)) plane-row ranges; chunk k
# needs only phi chunks 0..k (A: psum pair 0, B: pairs 1-2, C: pair 3)
GC = [((0, 18), (0, 17)), ((18, 50), (17, 49)), ((50, 66), (49, 64))]


# ---------------------------------------------------------------- host math


def _isclose_np(a, b):
    return np.abs(a - b) <= np.float32(ATOL) + np.float32(RTOL) * np.abs(b)


def _reference_np(x, weights, bias, positions):
    """Direct numpy port of the reference (fallback path)."""
    EPS = 1e-6
    Bn, Cn, Hn, Wn = x.shape
    On, _, Pn, KHn, KWn = weights.shape
    xp = np.pad(x, ((0, 0), (0, 0), (1, 1), (1, 1)))
    cols = [
        xp[:, :, i : i + Hn, j : j + Wn] for i in range(KHn) for j in range(KWn)
    ]
    pat = np.stack(cols, axis=2)
    v = pat.reshape(Bn, Cn, KHn * KWn, Hn * Wn).astype(np.float32)

    left, right = positions[:-1], positions[1:]
    denom = right - left
    denom = np.where(denom == 0, np.float32(EPS), denom)
    varc = (1.0 / denom).astype(np.float32)
    const = (-left * varc).astype(np.float32)

    m_first = _isclose_np(v, positions[0])
    m_last = _isclose_np(v, positions[-1])
    in_range = (~(m_first | m_last)) & (v >= positions[0]) & (v <= positions[-1])

    coeff = np.zeros(v.shape + (Pn,), np.float32)
    coeff[..., 0] += m_first.astype(np.float32)
    coeff[..., Pn - 1] += m_last.astype(np.float32)
    for p in range(Pn - 1):
        m = (in_range & (v >= positions[p]) & (v < positions[p + 1])).astype(
            np.float32
        )
        t = v * varc[p] + const[p]
        coeff[..., p] += m * (1.0 - t)
        coeff[..., p + 1] += m * t

    Wk = np.transpose(weights, (0, 1, 3, 4, 2)).reshape(On, Cn, KHn * KWn, Pn)
    ident = np.all(np.abs(Wk - 1.0) <= np.float32(ATOL + RTOL), axis=-1)
    Wk_eff = np.where(ident[..., None], np.float32(0.0), Wk)

    out = np.einsum("bcklp,ockp->bol", coeff, Wk_eff, optimize=True)
    out = out + np.einsum(
        "bckl,ock->bol", v, ident.astype(np.float32), optimize=True
    )
    out = out + bias[None, :, None]
    return out.reshape(Bn, On, Hn, Wn).astype(np.float32)


def _host_weights(weights, bias):
    """Fold the spline into two folded-weight blocks per tap.

    Returns (wstat [128, NS, O] f32, bias_eff [O] f32, ident_any).
    Stationary step s<3 (kw=s): rows = [-P(0,kw); Q(0,kw); -P(1,kw); Q(1,kw)]
    by 32-channel blocks; step s>=3 (kw=s-3): rows = [-P(2,kw); Q(2,kw); 0; 0]
    where P = W2-W3, Q = W4-W3 (so -P multiplies g2 and Q multiplies c4).
    """
    Wk = np.transpose(weights, (0, 1, 3, 4, 2)).reshape(O, C, K2, P)
    ident = np.all(np.abs(Wk - 1.0) <= np.float32(ATOL + RTOL), axis=-1)
    ident_any = bool(ident.any())
    W2 = Wk[:, :, :, 2].astype(np.float64)
    W3 = Wk[:, :, :, 3].astype(np.float64)
    W4 = Wk[:, :, :, 4].astype(np.float64)
    Pm = W2 - W3                      # [O, C, K2]
    Qm = W4 - W3
    # Partition layout is channel-interleaved (row 2c+0 = g2 lane of channel
    # c, row 2c+1 = c4 lane) so each Y gather is a single dma_start from the
    # (ch, plane, ...) ordered plane tile.  Steps 0-2 (kw = s) contract
    # Y = [g2; c4] x [rows r; rows r+1]: lower rows cover tap (0,kw), upper
    # tap (1,kw).  Steps 3-5 (kw = s-3) read Y at rows r+2 for tap (2,kw);
    # the upper 64 lanes get zero weights (K stays 128 for every matmul so
    # the PE never reconfigures its tile size).
    wstat = np.zeros((128, NS, O), np.float64)
    for kw in range(KW):
        wstat[0:64:2, kw] = -Pm[:, :, 0 * KW + kw].T
        wstat[1:64:2, kw] = Qm[:, :, 0 * KW + kw].T
        wstat[64:128:2, kw] = -Pm[:, :, 1 * KW + kw].T
        wstat[65:128:2, kw] = Qm[:, :, 1 * KW + kw].T
        wstat[0:64:2, KW + kw] = -Pm[:, :, 2 * KW + kw].T
        wstat[1:64:2, KW + kw] = Qm[:, :, 2 * KW + kw].T
    bias_eff = (bias.astype(np.float64) + W2.sum(axis=(1, 2))).astype(np.float32)
    return (
        np.ascontiguousarray(wstat.astype(np.float32)),
        bias_eff,
        ident_any,
    )


# ---------------------------------------------------------------- device IR


def _build_nc():
    import concourse.tile as tile
    from concourse import bacc, mybir

    f32 = mybir.dt.float32
    bf16 = mybir.dt.bfloat16
    Alu = mybir.AluOpType
    Act = mybir.ActivationFunctionType

    nc = bacc.Bacc("TRN2", target_bir_lowering=False, debug=False,
                   num_devices=NCORES)
    x_d = nc.dram_tensor("x", [IPC, C, H, W], bf16, kind="ExternalInput").ap()
    w_d = nc.dram_tensor("wstat", [128, NS, O], f32, kind="ExternalInput").ap()
    b_d = nc.dram_tensor("bias", [O, 1], f32, kind="ExternalInput").ap()
    o_d = nc.dram_tensor("out", [O, IPC, H, W], bf16, kind="ExternalOutput").ap()

    with tile.TileContext(nc) as tc:
        with (
            tc.tile_pool(name="const", bufs=1) as constp,
            tc.tile_pool(name="plane", bufs=1) as planep,
            tc.tile_pool(name="ybuf", bufs=2) as ybufp,
            tc.tile_pool(name="psum", bufs=1, space="PSUM") as psump,
            tc.tile_pool(name="osb", bufs=3) as osbp,
        ):
            # ---- x loads first (phi critical path) ----
            XF = planep.tile([IPC * C, H, W], bf16)
            for r0, r1 in RC:
                nc.sync.dma_start(
                    XF[:, r0:r1, :],
                    x_d[:, :, r0:r1, :].rearrange("i c h w -> (i c) h w"),
                )

            # PE warmup operand first on the gpsimd queue: the first warm
            # matmul gates on this memset
            zb = constp.tile([128, 512], bf16)
            nc.gpsimd.memset(zb[:], 0.0)

            # pull the ACT table load off the critical path (AP bias avoids
            # a framework const-tensor load in the prologue)
            negone = constp.tile([IPC * C, 1], f32)
            nc.gpsimd.memset(negone[:], -1.0)
            tiny = constp.tile([C, 1], f32)
            nc.gpsimd.memset(tiny[:], 0.0)
            nc.scalar.activation(
                tiny[:], tiny[:], Act.Relu, bias=negone[0:C, :], scale=1.0
            )
            warm_ctr = [0]

            def warm(nmm, gate=None):
                """Dummy matmuls (results never read).  gate, if given, is a
                bf16 AP used as the first mm's moving operand: the batch then
                starts only once that artifact exists, chaining PE busy-ness
                across the load/phi phase without racing ahead of it."""
                w = warm_ctr[0]
                warm_ctr[0] += 1
                pw = psump.tile(
                    [O, 512], f32, name=f"ps_warm{w}", tag=f"ps{w % 2}"
                )
                for j in range(nmm):
                    r = gate if (gate is not None and j == 0) else zb[:]
                    kp = r.shape[0]
                    nf = r.free_size()
                    nc.tensor.matmul(
                        pw[:, 0:nf], zb[0:kp, 0:128], r, start=True, stop=True
                    )

            # ---- weights ----
            w_sb = constp.tile([128, NS, O], f32)
            nc.sync.dma_start(w_sb[:], w_d[:])
            b_sb = constp.tile([O, 1], f32)
            nc.sync.dma_start(b_sb[:], b_d[:])

            # bridge PE busy-ness across the load/phi phase: one long batch
            # sized to span the lead-in (HAM re-throttles after ~3.4us idle),
            # plus a small batch gated on the weight cast as insurance
            warm(26)

            # ---- coefficient planes (both images, 64 partitions) ----
            # TP[:, 0] = g2 plane, TP[:, 1] = c4 plane (one tile so a single
            # dma_start gathers both planes into Y's partition blocks)
            TP = planep.tile([IPC * C, 2, HP, WP], bf16)
            for g in range(2):
                for strip in (
                    TP[:, g, 0, :],
                    TP[:, g, HP - 1, :],
                    TP[:, g, 1 : HP - 1, 0],
                    TP[:, g, 1 : HP - 1, WP - 1],
                ):
                    nc.gpsimd.memset(strip, 0.0)
            # weight cast on the otherwise-idle gpsimd engine: on vector it
            # would be scheduled behind all phi chunks, gating the first
            # real matmul on the whole phi phase
            w16 = constp.tile([128, NS, O], bf16)
            nc.gpsimd.tensor_copy(w16[:], w_sb[:])

            Ys = []
            for i in range(IPC):
                Y = ybufp.tile([128, HP, WP], bf16, name="Y", tag="Y")
                # singles (s>=3) read Y[64:128] rows up to HP with zero
                # weights; memset the never-written tail rows so nothing
                # reads uninitialized SBUF
                nc.gpsimd.memset(Y[64:128, H:HP, :], 0.0)
                Ys.append(Y)

            # phi and gathers interleaved per chunk; both images' gathers
            # fire as soon as their phi chunk lands.  The two triggers of a
            # chunk go to different queues (lower: sync, upper: scalar right
            # after that chunk's c4 op) so they issue in parallel.
            for (r0, r1), ((lo0, lo1), (hi0, hi1)) in zip(RC, GC):
                rr = slice(1 + r0, 1 + r1)
                nc.vector.tensor_scalar(
                    TP[:, 0, rr, 1 : W + 1], XF[:, r0:r1, :],
                    0.5, 2.0, Alu.min, Alu.mult,
                )
                nc.scalar.activation(
                    TP[:, 1, rr, 1 : W + 1], XF[:, r0:r1, :],
                    Act.Relu, bias=negone[:], scale=2.0,
                )
                for i in range(IPC):
                    s = slice(i * C, (i + 1) * C)
                    nc.sync.dma_start(
                        Ys[i][0:64, lo0:lo1, :], TP[s, :, lo0:lo1, :]
                    )
                    eng = nc.scalar if i == 0 else nc.sync
                    eng.dma_start(
                        Ys[i][64:128, hi0:hi1, :],
                        TP[s, :, hi0 + 1 : hi1 + 1, :],
                    )

            warm(2, gate=w16[0:64, 0, :])

            for i in range(IPC):
                Y = Ys[i]
                pss = [
                    psump.tile([O, RT * W], f32, name=f"ps{t}", tag=f"ps{t}")
                    for t in range(NT)
                ]
                # pair-outer: finish one PSUM bank pair (12 MMs), then drain
                # and store it while the next pair accumulates -- drains and
                # stores spread through the stream instead of piling up at
                # the end, and the next image's banks are long free
                for tp in range(NT // 2):
                    t0, t1 = 2 * tp, 2 * tp + 1
                    for s in range(NS):
                        kw = s % KW
                        for t in (t0, t1):
                            if s < KW:
                                rows = slice(t * RT, t * RT + RT)
                            else:
                                rows = slice(t * RT + 2, t * RT + 2 + RT)
                            rhs = Y[:, rows, kw : kw + W]
                            nc.tensor.matmul(
                                pss[t][:], w16[:, s, :], rhs,
                                start=(s == 0), stop=(s == NS - 1),
                            )
                    osb = osbp.tile([O, 2 * RT * W], bf16, name="osb")
                    nc.scalar.activation(
                        osb[:, 0 : RT * W], pss[t0][:], Act.Identity,
                        bias=b_sb[:, 0:1], scale=1.0,
                    )
                    nc.vector.tensor_scalar(
                        osb[:, RT * W : 2 * RT * W], pss[t1][:],
                        b_sb[:, 0:1], None, Alu.add,
                    )
                    last = i == IPC - 1 and tp == NT // 2 - 1
                    if last:
                        # split the final store across two trigger queues so
                        # both halves leave as soon as their drain finishes
                        for eng, h, t in ((nc.scalar, 0, t0), (nc.sync, 1, t1)):
                            eng.dma_start(
                                o_d[:, i, t * RT : t * RT + RT, :],
                                osb[:, h * RT * W : (h + 1) * RT * W]
                                .rearrange("o (r w) -> o r w", r=RT),
                            )
                    else:
                        nc.scalar.dma_start(
                            o_d[:, i, 2 * tp * RT : 2 * tp * RT + 2 * RT, :],
                            osb[:].rearrange("o (r w) -> o r w", r=2 * RT),
                        )
    nc.compile()
    return nc


# ---------------------------------------------------------------- entry


def _prep(inputs):
    x = np.ascontiguousarray(np.asarray(inputs["x"], dtype=np.float32))
    weights = np.ascontiguousarray(np.asarray(inputs["weights"], dtype=np.float32))
    bias = np.ascontiguousarray(np.asarray(inputs["bias"], dtype=np.float32))
    positions = np.ascontiguousarray(
        np.asarray(inputs["positions"], dtype=np.float32)
    )
    return x, weights, bias, positions


def _fast_path_ok(x, positions):
    expect = np.linspace(-1.0, 1.0, P, dtype=np.float32)
    return (
        x.shape == (B, C, H, W)
        and positions.shape == (P,)
        and np.array_equal(positions, expect)
        and float(x.min()) >= 0.0
        and float(x.max()) <= 1.0
    )


def kernel(**inputs):
    x, weights, bias, positions = _prep(inputs)
    if not _fast_path_ok(x, positions):
        return _reference_np(x, weights, bias, positions)

    wstat, bias_eff, ident_any = _host_weights(weights, bias)
    if ident_any:
        # identity-shortcut weights present: needs the raw-v plane; use the
        # exact fallback rather than a rarely-exercised device path
        return _reference_np(x, weights, bias, positions)

    from concourse.bass_utils import run_bass_kernel_spmd

    import ml_dtypes

    nc = _build_nc()
    bias2d = np.ascontiguousarray(bias_eff.reshape(O, 1))
    x16 = np.ascontiguousarray(x.astype(ml_dtypes.bfloat16))
    in_maps = [
        {"x": x16[i * IPC : (i + 1) * IPC],
         "wstat": wstat, "bias": bias2d}
        for i in range(NCORES)
    ]
    res = run_bass_kernel_spmd(nc, in_maps, core_ids=list(range(NCORES)))
    out = np.concatenate(
        [
            np.asarray(res.results[i]["out"])
            .astype(np.float32)
            .transpose(1, 0, 2, 3)
            for i in range(NCORES)
        ],
        axis=0,
    )
    return np.ascontiguousarray(out)


# ------------------------------------------------------------ dev utilities


def _run_sim(inputs):
    """CoreSim single-core run (images 0..IPC-1) for correctness debugging."""
    from concourse.bass_interp import CoreSim

    x, weights, bias, positions = _prep(inputs)
    assert _fast_path_ok(x, positions)
    wstat, bias_eff, ident_any = _host_weights(weights, bias)
    assert not ident_any
    import ml_dtypes

    nc = _build_nc()
    sim = CoreSim(nc)
    sim.tensor("x")[:] = x[:IPC].astype(ml_dtypes.bfloat16)
    sim.tensor("wstat")[:] = wstat
    sim.tensor("bias")[:] = bias_eff.reshape(O, 1)
    sim.simulate()
    return (
        np.array(sim.tensor("out")).astype(np.float32).transpose(1, 0, 2, 3)
    )
